# revision 1
# baseline (speedup 1.0000x reference)
"""Self-contained TRN2 Bass kernel for nn_EnhancedMultiheadAttention.

kernel(**inputs) takes the FULL unsharded inputs (x, Wq, bq, Wk, bk, Wv, bv,
Wo, bo as float32 numpy arrays), distributes the computation across 8
NeuronCores (tensor-parallel over heads: core c owns heads 2c, 2c+1), and
returns the full [2, 2048, 1024] float32 output.
"""

import sys

for _p in ("/opt/trn_rl_repo", "/root/.axon_site/_ro/trn_rl_repo"):
    if _p not in sys.path:
        sys.path.append(_p)



import numpy as np

import concourse.bass as bass
import concourse.mybir as mybir
import concourse.tile as tile
import bass_rust

F32 = mybir.dt.float32
F32R = mybir.dt.float32r

B, L, D = 2, 2048, 1024
H, DH = 16, 64
NCORES = 8
T = B * L                  # 4096 flattened tokens
TC = T // NCORES           # 512 tokens per core for the output slice
NKC = D // 128             # 8 contraction chunks of 128
CHUNK = 1024               # projection token-chunk width
NT = T // CHUNK            # 4 projection chunks
NJ = L // 128              # 16 key chunks of 128 per sequence
IB = 1024                  # query block width
NI = L // IB               # 2 query blocks per sequence
HPC = H // NCORES          # 2 heads per core


def split_excess_waits(nc, max_waits=1):
    """walrus's setupSyncWait rejects instructions with more than one wait
    condition on this compiler version; hoist extras onto preceding NoOps."""
    n_split = 0
    for f in nc.m.functions:
        for b in f.blocks:
            new_list = None
            for inst in list(b.instructions):
                si = inst.sync_info
                if si is None or len(si.on_wait) <= max_waits:
                    continue
                waits = list(si.on_wait)
                keep = waits[-max_waits:]
                excess = waits[:-max_waits]
                nops = []
                for j, w in enumerate(excess):
                    nop = mybir.InstNoOp(
                        name=f"I-wsplit-{inst.name}-{j}", ins=[], outs=[],
                        engine=inst.engine,
                    )
                    nop.sync_info = bass_rust.SyncInfo(on_wait=[w], on_update=[])
                    nops.append(nop)
                inst.sync_info = bass_rust.SyncInfo(
                    on_wait=keep, on_update=list(si.on_update)
                )
                if new_list is None:
                    new_list = list(b.instructions)
                pos = new_list.index(inst)
                new_list[pos:pos] = nops
                n_split += 1
            if new_list is not None:
                b.instructions = new_list
    return n_split


def build_nc(proj_mm="f32r", attn_mm="f32r", out_mm="f32r"):
    nc = bass.Bass("TRN2", target_bir_lowering=False, debug=False,
                   num_devices=NCORES)

    # f32r DRAM tensors take raw fp32 bytes from the host; the PE rounds
    # mantissas internally (TF32-like) at 4x the fp32 matmul rate.
    PD = F32R if proj_mm == "f32r" else F32
    OD = F32R if out_mm == "f32r" else F32
    xT = nc.dram_tensor("xT", [D, T], PD, kind="ExternalInput").ap()
    wq = nc.dram_tensor("wq", [D, 128], PD, kind="ExternalInput").ap()
    wk = nc.dram_tensor("wk", [D, 128], PD, kind="ExternalInput").ap()
    wv = nc.dram_tensor("wv", [D, 128], PD, kind="ExternalInput").ap()
    bq = nc.dram_tensor("bq", [128, 1], F32, kind="ExternalInput").ap()
    bk = nc.dram_tensor("bk", [128, 1], F32, kind="ExternalInput").ap()
    bv = nc.dram_tensor("bv", [128, 1], F32, kind="ExternalInput").ap()
    wo = nc.dram_tensor("wo", [D, D], OD, kind="ExternalInput").ap()
    bo = nc.dram_tensor("bo", [1, D], F32, kind="ExternalInput").ap()
    cosT = nc.dram_tensor("cosT", [128, L], F32, kind="ExternalInput").ap()
    sinT = nc.dram_tensor("sinT", [128, L], F32, kind="ExternalInput").ap()
    out = nc.dram_tensor("out", [TC, D], F32, kind="ExternalOutput").ap()

    with tile.TileContext(nc) as tc:
        _build_body(nc, tc, xT, wq, wk, wv, bq, bk, bv, wo, bo, cosT, sinT,
                    out, proj_mm, attn_mm, out_mm)

    split_excess_waits(nc)
    return nc


def _build_body(nc, tc, xT, wq, wk, wv, bq, bk, bv, wo, bo, cosT, sinT, out,
                proj_mm, attn_mm, out_mm):
    from contextlib import ExitStack

    ctx = ExitStack()
    with ctx:
        AD = F32R if attn_mm == "f32r" else F32
        PD = F32R if proj_mm == "f32r" else F32
        OD = F32R if out_mm == "f32r" else F32

        # ---------------- persistent tensors ----------------
        persist = ctx.enter_context(tc.tile_pool(name="persist", bufs=1))
        qt_sb = persist.tile([128, T], AD, tag="qt", name="qt")
        kt_sb = [persist.tile([128, T], AD, tag=f"kt{h}", name=f"kt{h}")
                 for h in range(HPC)]
        v_sb = [persist.tile([128, T // 128, DH + 1], AD, tag=f"v{h}",
                             name=f"v{h}")
                for h in range(HPC)]
        ident = persist.tile([128, 128], F32, tag="ident", name="ident")

        from concourse.masks import make_identity
        make_identity(nc, ident[:])

        # ones column of V (softmax denominator trick) + zero pads of K
        # (memset can't write f32r; write f32 and copy-convert)
        ones_col = persist.tile([128, 1], F32, tag="ones", name="ones")
        nc.gpsimd.memset(ones_col[:], 1.0)
        for h in range(HPC):
            nc.vector.tensor_copy(
                v_sb[h][:, :, DH:DH + 1],
                ones_col[:, :].to_broadcast((128, T // 128, 1)))
        zero_col = persist.tile([128, 1], F32, tag="zeros", name="zeros")
        nc.gpsimd.memset(zero_col[:], 0.0)
        nc.vector.tensor_copy(kt_sb[0][DH:128, :],
                              zero_col[DH:128, :].to_broadcast((DH, T)))
        nc.vector.tensor_copy(kt_sb[1][0:DH, :],
                              zero_col[0:DH, :].to_broadcast((DH, T)))

        wpool = ctx.enter_context(tc.tile_pool(name="wqkv", bufs=1))
        w_t = {}
        b_t = {}
        for name, wap, bap in (("q", wq, bq), ("k", wk, bk), ("v", wv, bv)):
            w_t[name] = wpool.tile([128, NKC, 128], PD, tag=f"w{name}",
                                   name=f"w{name}")
            nc.sync.dma_start(w_t[name][:],
                              wap.rearrange("(kc p) m -> p kc m", p=128))
            b_t[name] = wpool.tile([128, 1], F32, tag=f"b{name}",
                                   name=f"b{name}")
            nc.sync.dma_start(b_t[name][:], bap)

        # DRAM buffers for the collectives (one per local head, 64-row shards)
        dram = ctx.enter_context(tc.tile_pool(name="dram", bufs=1, space="DRAM"))
        a2a_in = [dram.tile([NCORES * DH, TC], OD, name=f"a2a_in{h}")
                  for h in range(HPC)]
        a2a_out = [dram.tile([NCORES * DH, TC], OD, name=f"a2a_out{h}")
                   for h in range(HPC)]

        xT3 = xT.rearrange("(kc p) t -> p kc t", p=128)

        # ---------------- phase A: projections + RoPE + V transpose ----------
        actx = ExitStack()
        xpool = actx.enter_context(tc.tile_pool(name="x", bufs=2))
        cspool = actx.enter_context(tc.tile_pool(name="cs", bufs=1))
        tmp = actx.enter_context(tc.tile_pool(name="ptmp", bufs=3))
        ppsum = actx.enter_context(tc.tile_pool(name="ppsum", bufs=2, space="PSUM"))
        vtpsum = actx.enter_context(tc.tile_pool(name="vtpsum", bufs=2, space="PSUM"))

        cos_l = cspool.tile([128, L], F32, tag="cos", name="cos")
        sin_l = cspool.tile([128, L], F32, tag="sin", name="sin")
        nc.sync.dma_start(cos_l[:], cosT[:])
        nc.sync.dma_start(sin_l[:], sinT[:])

        for i in range(NT):
            tsl = bass.ts(i, CHUNK)
            xt = xpool.tile([128, NKC, CHUNK], PD, tag="xchunk", name="xchunk")
            for kc in range(NKC):
                nc.sync.dma_start(xt[:, kc, :], xT3[:, kc, tsl])
            lsl = bass.ts(i % (L // CHUNK), CHUNK)
            cos_t = cos_l[:, lsl]
            sin_t = sin_l[:, lsl]

            for name in ("q", "k", "v"):
                ps = ppsum.tile([128, CHUNK], F32, tag="proj", name="proj")
                for kc in range(NKC):
                    for nh in range(CHUNK // 512):
                        nc.tensor.matmul(
                            ps[:, bass.ts(nh, 512)],
                            w_t[name][:, kc, :],
                            xt[:, kc, bass.ts(nh, 512)],
                            start=(kc == 0), stop=(kc == NKC - 1),
                        )
                raw = tmp.tile([128, CHUNK], F32, tag="raw", name="raw")
                nc.scalar.activation(raw[:], ps[:],
                                     mybir.ActivationFunctionType.Identity,
                                     bias=b_t[name][:])
                if name == "v":
                    # transpose [64,128] blocks into V ([token, d] layout)
                    for h in range(HPC):
                        for sub in range(CHUNK // 128):
                            jg = i * (CHUNK // 128) + sub
                            tp = vtpsum.tile([128, DH], F32, tag="vt", name="vt")
                            nc.tensor.transpose(
                                tp[:],
                                raw[bass.ds(h * DH, DH), bass.ts(sub, 128)],
                                ident[bass.ds(h * DH, DH), bass.ds(h * DH, DH)],
                            )
                            nc.vector.tensor_copy(v_sb[h][:, jg, 0:DH], tp[:])
                else:
                    shifted = tmp.tile([128, CHUNK], F32, tag="shift",
                                       name="shift")
                    for h in range(HPC):
                        o = h * DH
                        nc.gpsimd.dma_start(shifted[o:o + 32, :],
                                            raw[o + 32:o + 64, :])
                        nc.gpsimd.dma_start(shifted[o + 32:o + 64, :],
                                            raw[o:o + 32, :])
                    t1 = tmp.tile([128, CHUNK], F32, tag="t1", name="t1")
                    nc.vector.tensor_mul(t1[:], raw[:], cos_t)
                    nc.vector.tensor_mul(shifted[:], shifted[:], sin_t)
                    if name == "q":
                        nc.vector.tensor_add(qt_sb[:, tsl], t1[:], shifted[:])
                    else:
                        for h in range(HPC):
                            o = h * DH
                            nc.vector.tensor_add(
                                kt_sb[h][o:o + DH, tsl],
                                t1[o:o + DH, :], shifted[o:o + DH, :])

        actx.close()

        # Wo/bo loads overlap phase B (SBUF freed by phase A pools);
        # opool sits on the outer stack so it survives into phase C, and is
        # entered before the phase-B pools to keep pool push/pop LIFO.
        opool = ctx.enter_context(tc.tile_pool(name="oproj", bufs=1))
        wo_sb = opool.tile([128, NKC, D], OD, tag="wo", name="wo")
        bo_sb = opool.tile([128, D], F32, tag="bo", name="bo")
        ctx_sb = opool.tile([128, NKC, TC], OD, tag="ctxsb", name="ctxsb")

        # ---------------- phase B: attention (h outer, A2A per head) --------
        bctx = ExitStack()
        ppool = bctx.enter_context(tc.tile_pool(name="pT", bufs=4))
        npool = bctx.enter_context(tc.tile_pool(name="norm", bufs=2))
        cpool = bctx.enter_context(tc.tile_pool(name="ctx", bufs=2))
        stpsum = bctx.enter_context(tc.tile_pool(name="stpsum", bufs=2, space="PSUM"))
        avpsum = bctx.enter_context(tc.tile_pool(name="avpsum", bufs=2, space="PSUM"))
        ndram = bctx.enter_context(tc.tile_pool(name="ndram", bufs=2, space="DRAM"))

        for b in range(B):
            co = b * L
            if b == 1:
                # Wo/bo loads deferred here so they don't steal phase-A DMA
                # bandwidth from the x stream; plenty of time before phase C.
                nc.sync.dma_start(wo_sb[:],
                                  wo.rearrange("(kc p) n -> p kc n", p=128))
                nc.sync.dma_start(bo_sb[:], bo.to_broadcast((128, D)))
            for h in range(HPC):
                for ib in range(NI):
                    av = avpsum.tile([128, IB], F32, tag="av", name="av")
                    for jc in range(NJ):
                        st = stpsum.tile([128, IB], F32, tag="st", name="st")
                        for nh in range(IB // 512):
                            nc.tensor.matmul(
                                st[:, bass.ts(nh, 512)],
                                kt_sb[h][:, bass.ds(co + jc * 128, 128)],
                                qt_sb[:, bass.ds(co + ib * IB + nh * 512, 512)],
                                start=True, stop=True,
                            )
                        pt = ppool.tile([128, IB], AD, tag="pt", name="pt")
                        nc.scalar.activation(pt[:], st[:],
                                             mybir.ActivationFunctionType.Exp,
                                             scale=float(DH) ** -0.5)
                        for nh in range(IB // 512):
                            nc.tensor.matmul(
                                av[0:DH + 1, bass.ts(nh, 512)],
                                v_sb[h][:, b * NJ + jc, :],
                                pt[:, bass.ts(nh, 512)],
                                start=(jc == 0), stop=(jc == NJ - 1),
                            )
                    # normalize by 1/l (l = row DH of av); partition-broadcast
                    # via a DRAM bounce row (SBUF step-0 APs are rejected).
                    recip = npool.tile([1, IB], F32, tag="recip", name="recip")
                    nc.vector.reciprocal(recip[:], av[DH:DH + 1, :])
                    rrow = ndram.tile([1, IB], F32, tag="rrow", name="rrow")
                    nc.sync.dma_start(rrow[:], recip[:])
                    rb = npool.tile([DH, IB], F32, tag="rb", name="rb")
                    nc.sync.dma_start(rb[:], rrow[0:1, :].to_broadcast((DH, IB)))
                    cx = cpool.tile([DH, IB], OD, tag="cx", name="cx")
                    nc.vector.tensor_mul(cx[:], av[0:DH, :], rb[:])
                    for half in range(IB // TC):
                        g = (b * L + ib * IB + half * TC) // TC
                        nc.sync.dma_start(
                            a2a_in[h][bass.ds(g * DH, DH), :],
                            cx[:, bass.ts(half, TC)])

                if b == 1:
                    # head h complete across both batches: redistribute it
                    nc.gpsimd.collective_compute(
                        "AllToAll",
                        mybir.AluOpType.bypass,
                        replica_groups=[list(range(NCORES))],
                        ins=[a2a_in[h][:]],
                        outs=[a2a_out[h][:]],
                    )
                    for kc in range(NKC):
                        nc.gpsimd.dma_start(
                            ctx_sb[bass.ds(h * DH, DH), kc, :],
                            a2a_out[h][bass.ts(kc, DH), :])
                    if h == HPC - 1:
                        # PE warm-keeper through the A2A wait: throwaway
                        # matmuls on the last cx tile (results unused).
                        wps = stpsum.tile([128, 512], F32, tag="st",
                                          name="warm")
                        for wr in range(12):
                            nc.tensor.matmul(
                                wps[:], cx[:, bass.ts(wr % 4, 128)],
                                cx[:, 0:512], start=True, stop=True)

        bctx.close()

        # ---------------- phase C: output projection ----------------
        ostage = ctx.enter_context(tc.tile_pool(name="ostage", bufs=2))
        opsum = ctx.enter_context(tc.tile_pool(name="opsum", bufs=3, space="PSUM"))

        for tch in range(TC // 128):
            pss = [opsum.tile([128, 512], F32, tag=f"ops{nh}", name=f"ops{nh}")
                   for nh in range(2)]
            for kc in range(NKC):
                for nh in range(2):
                    nc.tensor.matmul(
                        pss[nh][:],
                        ctx_sb[:, kc, bass.ts(tch, 128)],
                        wo_sb[:, kc, bass.ts(nh, 512)],
                        start=(kc == 0), stop=(kc == NKC - 1),
                    )
            for nh in range(2):
                ot = ostage.tile([128, 512], F32, tag="ot", name="ot")
                nc.vector.tensor_add(ot[:], pss[nh][:], bo_sb[:, bass.ts(nh, 512)])
                nc.sync.dma_start(out[bass.ts(tch, 128), bass.ts(nh, 512)], ot[:])


# ---------------- host-side sharding / unsharding ----------------

def rope_cos_sin_np(seq_len, d_head):
    inv_freq = 1.0 / (10000.0 ** (np.arange(0, d_head, 2, dtype=np.float32) / d_head))
    t = np.arange(seq_len, dtype=np.float32)
    freqs = np.einsum("i,j->ij", t, inv_freq).astype(np.float32)
    emb = np.concatenate((freqs, freqs), axis=-1)
    return np.cos(emb).astype(np.float32), np.sin(emb).astype(np.float32)


def make_in_maps(x, Wq, bq, Wk, bk, Wv, bv, Wo, bo):
    xT = np.ascontiguousarray(x.reshape(T, D).T)

    cos, sin = rope_cos_sin_np(L, DH)          # [L, 64]
    cosT = cos.T                               # [64, L]
    sinT = sin.T
    sgn = np.where(np.arange(DH) < DH // 2, -1.0, 1.0).astype(np.float32)
    sinT_signed = sinT * sgn[:, None]
    # stack 2 heads on partitions; batches reuse the same positions
    cosT_full = np.ascontiguousarray(np.tile(cosT, (HPC, 1)))      # [128, 2048]
    sinT_full = np.ascontiguousarray(np.tile(sinT_signed, (HPC, 1)))

    wo_full = np.ascontiguousarray(Wo)
    bo_row = np.ascontiguousarray(bo.reshape(1, D))

    in_maps = []
    for c in range(NCORES):
        sl = slice(c * 128, (c + 1) * 128)
        in_maps.append({
            "xT": xT,
            "wq": np.ascontiguousarray(Wq[:, sl]),
            "wk": np.ascontiguousarray(Wk[:, sl]),
            "wv": np.ascontiguousarray(Wv[:, sl]),
            "bq": np.ascontiguousarray(bq[sl].reshape(128, 1)),
            "bk": np.ascontiguousarray(bk[sl].reshape(128, 1)),
            "bv": np.ascontiguousarray(bv[sl].reshape(128, 1)),
            "wo": wo_full,
            "bo": bo_row,
            "cosT": cosT_full,
            "sinT": sinT_full,
        })
    return in_maps


def assemble_output(results):
    parts = [results[c]["out"] for c in range(NCORES)]
    return np.concatenate(parts, axis=0).reshape(B, L, D).astype(np.float32)


_CACHE = {}


def _get_runner():
    """Build the Bass program and a cached jitted SPMD executor once.

    Mirrors bass2jax.run_bass_via_pjrt's multi-core path, but keeps the
    jitted shard_map callable alive so repeat kernel() calls skip retracing.
    """
    if "runner" in _CACHE:
        return _CACHE["runner"]

    import jax
    import numpy as _np
    from jax.sharding import Mesh, PartitionSpec
    from jax.experimental.shard_map import shard_map
    from concourse import bass2jax, mybir as _mybir

    nc = build_nc(proj_mm="f32r", attn_mm="f32r", out_mm="f32r")
    bass2jax.install_neuronx_cc_hook()

    partition_name = (nc.partition_id_tensor.name
                      if nc.partition_id_tensor else None)
    in_names, out_names, out_avals, zero_shapes = [], [], [], []
    for alloc in nc.m.functions[0].allocations:
        if not isinstance(alloc, _mybir.MemoryLocationSet):
            continue
        name = alloc.memorylocations[0].name
        if alloc.kind == "ExternalInput":
            if name != partition_name:
                in_names.append(name)
        elif alloc.kind == "ExternalOutput":
            shape = tuple(alloc.tensor_shape)
            dtype = _mybir.dt.np(alloc.dtype)
            out_names.append(name)
            out_avals.append(jax.core.ShapedArray(shape, dtype))
            zero_shapes.append((shape, dtype))
    n_params = len(in_names)
    n_outs = len(out_avals)
    all_in_names = list(in_names) + list(out_names)
    if partition_name is not None:
        all_in_names.append(partition_name)
    donate = tuple(range(n_params, n_params + n_outs))

    def _body(*args):
        operands = list(args)
        if partition_name is not None:
            operands.append(bass2jax.partition_id_tensor())
        outs = bass2jax._bass_exec_p.bind(
            *operands,
            out_avals=tuple(out_avals),
            in_names=tuple(all_in_names),
            out_names=tuple(out_names),
            lowering_input_output_aliases=(),
            sim_require_finite=True,
            sim_require_nnan=True,
            nc=nc,
        )
        return tuple(outs)

    devices = jax.devices()[:NCORES]
    mesh = Mesh(_np.asarray(devices), ("core",))
    in_specs = (PartitionSpec("core"),) * (n_params + n_outs)
    out_specs = (PartitionSpec("core"),) * n_outs
    sharded = jax.jit(
        shard_map(_body, mesh=mesh, in_specs=in_specs, out_specs=out_specs,
                  check_rep=False),
        donate_argnums=donate,
        keep_unused=True,
    )

    def run(in_maps):
        per_core = [[_np.asarray(m[name]) for name in in_names]
                    for m in in_maps]
        concat_in = [
            _np.concatenate([per_core[c][i] for c in range(NCORES)], axis=0)
            for i in range(n_params)
        ]
        concat_zeros = [
            _np.zeros((NCORES * s[0], *s[1:]), dt) for s, dt in zero_shapes
        ]
        out_arrs = sharded(*concat_in, *concat_zeros)
        return [
            {name: _np.asarray(out_arrs[i]).reshape(
                NCORES, *out_avals[i].shape)[c]
             for i, name in enumerate(out_names)}
            for c in range(NCORES)
        ]

    _CACHE["runner"] = run
    return run


def kernel(**inputs):
    run = _get_runner()
    in_maps = make_in_maps(**{k: np.asarray(v, dtype=np.float32)
                              for k, v in inputs.items()})
    return assemble_output(run(in_maps))

